# revision 10
# baseline (speedup 1.0000x reference)
"""Trainium2 Bass kernel for nn_DZSpecimenClfToy (v5).

Reference computation (per batch item b, B=8, one NeuronCore each):
  1. tv = bilinear_resize(topview[b], (3,64,64) -> (3,4,4))
  2. coords = sigmoid(tv.flat @ W1.T + b1).reshape(N,2)       # N=4096
  3. tl = coords*2043; 5x5x3 bilinear support per patch
  4. out[b] = bilinear_crops.flat @ W2.T + b2                 # [2]

Sharding: data-parallel over batch across 8 cores; weights replicated.

Host re-lays the search view as a cell table svc[r*2048+c] = 16 bf16
(rows r..r+4 of column c, 15 values + pad), so a patch at (r0,c0) is ONE
contiguous 79-bf16 run at cell index r0*2048+c0 (< 2^23: float magic
rounding gives the exact int index; no div/mod needed).

The HW indirect DMA supports one offset per partition per instruction,
so the gather is 32 x [128 offsets] instructions serialized on the
GpSimd Q7 (~1.4us each) - the dominant wall.  All other compute is
pipelined UNDER it: patches are processed in uneven groups (small first
group so gathers start early, small last group for a short tail); while
later groups are being gathered, the bf16 bilinear combine + classifier
partial dot-products for earlier groups run on the Vector engine.

floor() is computed as round_half_even(x - 0.5) in one fused op; the
half-integer cases land on a neighbouring cell with fraction 0/1, which
bilinear interpolation maps to the same value.  Fractions are produced
NEGATED (one fused op) and the combine uses S0-S1 / T0-T1 differences
to compensate.

Precision: coords matvec in f32 (bf16 shifts patches ~0.1px: fails);
patch data / combine / W2 in bf16 (~5e-3 rel err vs 2e-2 budget).
"""
import functools
from contextlib import ExitStack

import numpy as np
import ml_dtypes

import concourse.bass as bass
import concourse.tile as tile
from concourse import bacc, mybir
import concourse.bass_utils as bass_utils
from concourse.bass import IndirectOffsetOnAxis

F32 = mybir.dt.float32
BF16 = mybir.dt.bfloat16
I32 = mybir.dt.int32
ALU = mybir.AluOpType
ACT = mybir.ActivationFunctionType
AX = mybir.AxisListType

B = 8          # batch == number of cores
H = W = 2048   # search view height/width
N = 4096       # patches per item
PS = 4         # patch size
NCLS = 2       # classes
P = 128        # partitions
TPP = N // P   # patches per partition = 32

R = H - PS     # 2044 rows of 5-row cells (r0 in [0, 2043])
CELL = 16      # bf16 per (row, col) cell: 5 rows x 3 ch + 1 pad
SEG = 79       # gathered bf16 per patch: max offset 4*16+4*3+2 = 78
SEGP = 80      # SBUF stride per patch segment
MAGIC = 8388608.0      # 2**23
# Fused floor: tl2 = tl+2 in [2, 2045]; round_half_even(tl2 - 0.5) via
# +(2^23-0.5) keeps every sum >= 2^23+1.5 (integer-granularity regime).
MAGICH = MAGIC - 0.5              # exact in f32 (below 2^23)
BIASF = MAGIC + 2.0               # r0b = BIASF + floor(tl)
C1 = BIASF * float(W)             # 2^34 + 2^12, exact in f32
NP_BF16 = ml_dtypes.bfloat16

CHUNKS = [6, 10, 10, 6]  # patches per partition per pipeline group
assert sum(CHUNKS) == TPP


def build_program(num_devices: int, svh: int, svw: int):
    pad = float(svh - 1 - PS)  # 2043
    assert svh == H and svw == W, (svh, svw)

    nc = bacc.Bacc("TRN2", target_bir_lowering=False, debug=False,
                   enable_asserts=False, num_devices=num_devices)

    tvs = nc.dram_tensor("tvs", [1, 1536], F32, kind="ExternalInput").ap()
    svc = nc.dram_tensor("svc", [R * W, CELL], BF16, kind="ExternalInput").ap()
    w1 = nc.dram_tensor("W1k", [P, 64 * 48], F32, kind="ExternalInput").ap()
    b1 = nc.dram_tensor("b1k", [P, 64], F32, kind="ExternalInput").ap()
    w2 = nc.dram_tensor("W2k", [P, NCLS * 1536], BF16, kind="ExternalInput").ap()
    b2 = nc.dram_tensor("b2k", [1, NCLS], F32, kind="ExternalInput").ap()
    out = nc.dram_tensor("out", [1, NCLS], F32, kind="ExternalOutput").ap()

    JC0 = 2 * CHUNKS[0]  # coord lanes in the first group

    with tile.TileContext(nc) as tc:
        with ExitStack() as ctx:
            pool = ctx.enter_context(tc.tile_pool(name="main", bufs=1))

            # ---- input DMAs. sync ring: W1 group-0 slice, then the rest of
            # W1.  scalar ring: topview (tiny, gates everything), W2, biases.
            Asb = pool.tile([1, 1536], F32)
            nc.scalar.dma_start(Asb[:], tvs)
            W1sb = pool.tile([P, 64 * 48], F32)
            nc.sync.dma_start(W1sb[:, 0:JC0 * 48], w1[:, 0:JC0 * 48])
            nc.sync.dma_start(W1sb[:, JC0 * 48:], w1[:, JC0 * 48:])
            b1sb = pool.tile([P, 64], F32)
            nc.scalar.dma_start(b1sb[:], b1)
            W2sb = pool.tile([P, NCLS * 1536], BF16)
            nc.scalar.dma_start(W2sb[:], w2)
            b2sb = pool.tile([1, NCLS], F32)
            nc.scalar.dma_start(b2sb[:], b2)

            # ---- topview 64x64 -> 4x4 resize on partition 0, then
            # broadcast (x0.25 folded into W1k on the host)
            V = pool.tile([1, 768], F32)
            A4 = Asb[:].rearrange("p (ck r w) -> p ck r w", ck=12, r=2)
            nc.vector.tensor_add(V[:].rearrange("p (ck w) -> p ck w", ck=12),
                                 A4[:, :, 0, :], A4[:, :, 1, :])
            flatF = pool.tile([1, 48], F32)
            V4 = V[:].rearrange("p (ck g s) -> p ck g s", ck=12, g=4)
            nc.vector.tensor_add(flatF[:].rearrange("p (ck g) -> p ck g", ck=12),
                                 V4[:, :, :, 7], V4[:, :, :, 8])
            flatb = pool.tile([P, 48], F32)
            nc.gpsimd.partition_broadcast(flatb[:], flatF[:], channels=P)

            # ---- coords + gather index, in pipeline groups -----------------
            mul1 = pool.tile([P, 64 * 48], F32)
            pre = pool.tile([P, 64], F32)
            sg = pool.tile([P, 64], F32)
            tl = pool.tile([P, 64], F32)
            r0b = pool.tile([P, 64], F32)   # 2^23 + floor(tl)
            nf = pool.tile([P, 64], F32)    # -fraction
            t1 = pool.tile([P, TPP], F32)
            idxm = pool.tile([P, TPP], F32)
            idxi = pool.tile([P, TPP], I32)
            frx = pool.tile([P, TPP * 12], BF16)   # -fr expanded x12
            fcx = pool.tile([P, TPP * 12], BF16)   # -fc expanded x12

            Schunks = []
            for k in range(len(CHUNKS)):
                Sk = pool.tile([P, CHUNKS[k] * SEGP], BF16, tag=f"S{k}")
                Schunks.append(Sk)

            tbase = 0
            for k, TPC in enumerate(CHUNKS):
                JC = 2 * TPC
                js = slice(2 * tbase, 2 * tbase + JC)
                ts = slice(tbase, tbase + TPC)
                m1v = mul1[:, 2 * tbase * 48:(2 * tbase + JC) * 48] \
                    .rearrange("p (j c) -> p j c", j=JC)
                nc.vector.tensor_mul(
                    m1v, W1sb[:, 2 * tbase * 48:(2 * tbase + JC) * 48]
                    .rearrange("p (j c) -> p j c", j=JC),
                    flatb[:].unsqueeze(1).to_broadcast((P, JC, 48)))
                nc.vector.reduce_sum(pre[:, js].unsqueeze(2), m1v, axis=AX.X)
                nc.vector.tensor_add(pre[:, js], pre[:, js], b1sb[:, js])
                nc.scalar.activation(sg[:, js], pre[:, js], ACT.Sigmoid)
                # tl2 = tl + 2 in [2, 2045]
                nc.vector.tensor_scalar(tl[:, js], sg[:, js], pad, 2.0,
                                        op0=ALU.mult, op1=ALU.add)
                # r0b = BIASF + floor(tl): round_half_even(tl2 - 0.5) + 2^23
                nc.vector.tensor_scalar_add(r0b[:, js], tl[:, js], MAGICH)
                # nf = (r0b - 2^23) - tl2 = floor(tl) - tl = -fraction
                nc.vector.scalar_tensor_tensor(nf[:, js], r0b[:, js], MAGIC,
                                               tl[:, js],
                                               op0=ALU.subtract,
                                               op1=ALU.subtract)
                # t1 = (r0b_r - BIASF)*2048 = r0_r*2048
                r0v = r0b[:, js].rearrange("p (t two) -> p t two", two=2)
                nc.vector.tensor_scalar(t1[:, ts], r0v[:, :, 0], float(W),
                                        -C1, op0=ALU.mult, op1=ALU.add)
                # idxm = (r0b_c - 2) + t1 = 2^23 + r0_r*2048 + r0_c
                nc.vector.scalar_tensor_tensor(idxm[:, ts], r0v[:, :, 1],
                                               2.0, t1[:, ts],
                                               op0=ALU.subtract, op1=ALU.add)
                nc.vector.tensor_single_scalar(idxi[:, ts],
                                               idxm[:, ts].bitcast(I32),
                                               0x007FFFFF, op=ALU.bitwise_and)
                nfv = nf[:, js].rearrange("p (t two) -> p t two", two=2)
                nc.vector.tensor_scalar_mul(
                    frx[:, tbase * 12:(tbase + TPC) * 12]
                    .rearrange("p (t e) -> p t e", t=TPC),
                    nfv[:, :, 0:1].to_broadcast((P, TPC, 12)), 1.0)
                nc.vector.tensor_scalar_mul(
                    fcx[:, tbase * 12:(tbase + TPC) * 12]
                    .rearrange("p (t e) -> p t e", t=TPC),
                    nfv[:, :, 1:2].to_broadcast((P, TPC, 12)), 1.0)

                # gathers for this group: TPC instructions, 128 offsets each
                S = Schunks[k]
                for t in range(TPC):
                    tg = tbase + t
                    nc.gpsimd.indirect_dma_start(
                        out=S[:, t * SEGP:t * SEGP + SEG],
                        out_offset=None,
                        in_=svc,
                        in_offset=IndirectOffsetOnAxis(
                            ap=idxi[:, tg:tg + 1], axis=0),
                    )
                tbase += TPC

            # ---- per-group combine + classifier (overlaps later gathers) --
            TPCmax = max(CHUNKS)
            D1 = pool.tile([P, TPCmax * 60], BF16)
            M1 = pool.tile([P, TPCmax * 60], BF16)
            T = pool.tile([P, TPCmax * 60], BF16)
            D2 = pool.tile([P, TPCmax * 48], BF16)
            M2 = pool.tile([P, TPCmax * 48], BF16)
            U = pool.tile([P, TPCmax * 48], BF16)
            Pm = pool.tile([P, TPCmax * 48], BF16)
            r2all = pool.tile([P, NCLS * len(CHUNKS)], F32)
            ppool = ctx.enter_context(tc.tile_pool(name="ps", bufs=1,
                                                   space="PSUM"))
            ones = pool.tile([P, 1], F32)
            nc.vector.memset(ones[:], 1.0)

            tbase = 0
            for k, TPC in enumerate(CHUNKS):
                S = Schunks[k]
                Sc = S[:].rearrange("p (t d e) -> p t d e", t=TPC, d=5)
                S0 = Sc[:, :, :, 0:12]
                S1 = Sc[:, :, :, 3:15]
                frb = frx[:, tbase * 12:(tbase + TPC) * 12] \
                    .rearrange("p (t e) -> p t e", t=TPC).unsqueeze(2) \
                    .to_broadcast((P, TPC, 5, 12))
                fcb = fcx[:, tbase * 12:(tbase + TPC) * 12] \
                    .rearrange("p (t e) -> p t e", t=TPC).unsqueeze(2) \
                    .to_broadcast((P, TPC, 4, 12))
                # T = S0 + fr*(S1-S0) = S0 + nf*(S0-S1)
                D1v = D1[:, 0:TPC * 60].rearrange("p (t d e) -> p t d e", t=TPC, d=5)
                nc.vector.tensor_sub(D1v, S0, S1)
                M1v = M1[:, 0:TPC * 60].rearrange("p (t d e) -> p t d e", t=TPC, d=5)
                nc.vector.tensor_mul(M1v, D1v, frb)
                Tv = T[:, 0:TPC * 60].rearrange("p (t d e) -> p t d e", t=TPC, d=5)
                nc.vector.tensor_add(Tv, M1v, S0)
                T0 = Tv[:, :, 0:4, :]
                T1 = Tv[:, :, 1:5, :]
                D2v = D2[:, 0:TPC * 48].rearrange("p (t d e) -> p t d e", t=TPC, d=4)
                nc.vector.tensor_sub(D2v, T0, T1)
                M2v = M2[:, 0:TPC * 48].rearrange("p (t d e) -> p t d e", t=TPC, d=4)
                nc.vector.tensor_mul(M2v, D2v, fcb)
                Uv = U[:, 0:TPC * 48]
                nc.vector.tensor_add(
                    Uv.rearrange("p (t d e) -> p t d e", t=TPC, d=4), M2v, T0)
                for c in range(NCLS):
                    nc.vector.tensor_mul(
                        Pm[:, 0:TPC * 48], Uv,
                        W2sb[:, c * 1536 + tbase * 48:
                             c * 1536 + (tbase + TPC) * 48])
                    nc.vector.reduce_sum(
                        r2all[:, k * NCLS + c:k * NCLS + c + 1].unsqueeze(2),
                        Pm[:, 0:TPC * 48].unsqueeze(1), axis=AX.X)
                tbase += TPC

            # ---- final: sum group partials, partition-reduce, bias, store -
            r2 = pool.tile([P, NCLS], F32)
            r2v = r2all[:].rearrange("p (k c) -> p k c", k=len(CHUNKS))
            nc.vector.reduce_sum(r2[:].unsqueeze(1),
                                 r2v.rearrange("p k c -> p c k"), axis=AX.X)
            osum = ppool.tile([1, NCLS], F32)
            nc.tensor.matmul(out=osum[:], lhsT=ones[:], rhs=r2[:],
                             start=True, stop=True)
            ofin = pool.tile([1, NCLS], F32)
            nc.vector.tensor_add(ofin[:], osum[:], b2sb[:])
            nc.sync.dma_start(out, ofin[:])

    nc.compile()
    return nc


@functools.lru_cache(maxsize=2)
def _compiled(num_devices: int, svh: int, svw: int):
    return build_program(num_devices, svh, svw)


def cell_layout(img: np.ndarray) -> np.ndarray:
    """[2048, 2048, 3] f32 -> [2044*2048, 16] bf16 cell table."""
    sw = np.lib.stride_tricks.sliding_window_view(img, 5, axis=0)  # [2044,2048,3,5]
    cells = sw.transpose(0, 1, 3, 2).reshape(R, W, 15)             # (row, ch)
    buf = np.zeros((R, W, CELL), dtype=NP_BF16)
    buf[:, :, :15] = cells.astype(NP_BF16)
    return buf.reshape(R * W, CELL)


def permute_w2(W2: np.ndarray) -> np.ndarray:
    """(n, i, j, c) -> (n, j, i, c), then [p, (cls, t*48+x)] bf16."""
    w = W2.reshape(NCLS, N, PS, PS, 3).transpose(0, 1, 3, 2, 4)
    w = w.reshape(NCLS, P, TPP * 48).transpose(1, 0, 2)
    return np.ascontiguousarray(w.reshape(P, NCLS * 1536)).astype(NP_BF16)


def select_tv(tv: np.ndarray) -> np.ndarray:
    """[3,64,64] -> [1, 1536] rows {7,8},{23,24},{39,40},{55,56}."""
    sel = tv[:, (7, 8, 23, 24, 39, 40, 55, 56), :]
    return np.ascontiguousarray(sel.reshape(3, 4, 2, 64)).reshape(1, 1536)


def make_in_maps(topview, search_views, W1, b1, W2, b2):
    W1k = np.ascontiguousarray(
        (0.25 * np.asarray(W1, np.float32)).reshape(P, 64 * 48))
    b1k = np.ascontiguousarray(np.asarray(b1, np.float32).reshape(P, 64))
    W2k = permute_w2(np.ascontiguousarray(W2, np.float32))
    b2k = np.ascontiguousarray(np.asarray(b2, np.float32).reshape(1, NCLS))
    return [{
        "tvs": select_tv(np.ascontiguousarray(topview[i], np.float32)),
        "svc": cell_layout(np.ascontiguousarray(search_views[i], np.float32)),
        "W1k": W1k, "b1k": b1k, "W2k": W2k, "b2k": b2k,
    } for i in range(topview.shape[0])]


def kernel(topview, search_views, W1, b1, W2, b2, svh, svw):
    svh, svw = int(svh), int(svw)
    nc = _compiled(B, svh, svw)
    in_maps = make_in_maps(topview, search_views, W1, b1, W2, b2)
    res = bass_utils.run_bass_kernel_spmd(nc, in_maps, core_ids=list(range(B)))
    return np.concatenate([res.results[i]["out"] for i in range(B)], axis=0)
